# revision 1
# baseline (speedup 1.0000x reference)
# Trainium2 Bass kernel for DenseBipartiteGAT (B=8, N=1024, C=256, H=4, D=64).
#
# Math: scores[t,s,h] = lrelu(a_tgt[t,h] + a_src[s,h], 0.2), masked softmax over s,
#       out[t] = sum_s attn * h_src.
# Key factorization: exp(lrelu(u+v)) = exp(u)exp(v) if u+v>=0 else exp(.2u)exp(.2v).
# With M1 = edge_mask * [u+v>=0] (a 0/1 fp16 matrix) and em = edge_mask:
#   num = E1[t]*(M1^T @ F1h) + E2[t]*((em^T @ F2h) - (M1^T @ F2h))
# and dividing num/den cancels E2, leaving r[t] = exp(0.8*u[t]) as the only
# target-side scale: out = (r*A1 + G - A2) / (r*A1d + Gd - A2d + eps).
# So the only O(N^2) elementwise work is: em = (adj != 0), P = [u+v>=0],
# M1 = P*em  -- everything else is matmuls on the PE.
#
# Sharding: data-parallel over batch B across the 8 cores (1 batch element each).

import hashlib
import os
import shutil

import numpy as np

B, N, C, H, D = 8, 1024, 256, 4, 64
NT = N // 128  # 8 tiles of 128 along s or t
EPS = 1e-12

_CACHED = {}


def _install_neff_cache():
    """Content-addressed NEFF cache: walrus compile is ~8min, cache by BIR hash."""
    import concourse.bass2jax as b2j
    import concourse.bass_utils as bu

    if getattr(b2j, "_neff_cache_installed", False):
        return
    cache_dir = os.environ.get("NEFF_CACHE_DIR", "/tmp/neff_cache")
    os.makedirs(cache_dir, exist_ok=True)
    orig = bu.compile_bir_kernel

    def cached_compile(bir_json: bytes, tmpdir: str, neff_name="file.neff") -> str:
        key = hashlib.sha256(bir_json).hexdigest()
        cpath = os.path.join(cache_dir, f"{key}.neff")
        opath = os.path.join(tmpdir, neff_name)
        if os.path.exists(cpath):
            shutil.copy(cpath, opath)
            return opath
        neff = orig(bir_json, tmpdir, neff_name)
        try:
            shutil.copy(neff, cpath)
        except OSError:
            pass
        return neff

    bu.compile_bir_kernel = cached_compile
    b2j.compile_bir_kernel = cached_compile
    b2j._neff_cache_installed = True


def build_nc():
    """Build the Bass program (one core's work; SPMD across 8 cores)."""
    import concourse.bass as bass
    import concourse.tile as tile
    import concourse.mybir as mybir
    from concourse import bacc
    from concourse.bass import ts, ds

    f32 = mybir.dt.float32
    f16 = mybir.dt.float16
    Alu = mybir.AluOpType
    Act = mybir.ActivationFunctionType

    nc = bacc.Bacc("TRN2", target_bir_lowering=False, debug=False, num_devices=B)

    xsT = nc.dram_tensor("xsT", (C, N), f32, kind="ExternalInput").ap()
    xtT = nc.dram_tensor("xtT", (C, N), f32, kind="ExternalInput").ap()
    adj = nc.dram_tensor("adj", (N, N), f32, kind="ExternalInput").ap()
    maskp = nc.dram_tensor("maskp", (128, NT), f32, kind="ExternalInput").ap()
    wes = nc.dram_tensor("wes", (C, 260), f32, kind="ExternalInput").ap()
    wbt = nc.dram_tensor("wbt", (C, 4), f32, kind="ExternalInput").ap()
    biasrow = nc.dram_tensor("biasrow", (1, 256), f32, kind="ExternalInput").ap()
    out = nc.dram_tensor("out", (N, 256), f32, kind="ExternalOutput").ap()

    with tile.TileContext(nc) as tc:
        with (
            tc.tile_pool(name="singles", bufs=1) as singles,
            tc.tile_pool(name="psum", bufs=8, space="PSUM") as psum_pool,
            tc.tile_pool(name="adjs", bufs=3) as adj_pool,
            tc.tile_pool(name="em16", bufs=3) as em16_pool,
            tc.tile_pool(name="emT", bufs=NT) as emT_pool,
            tc.tile_pool(name="rtile", bufs=NT) as r_pool,
            tc.tile_pool(name="asb", bufs=NT) as a_pool,
            tc.tile_pool(name="fex", bufs=2) as f_pool,
            tc.tile_pool(name="m1", bufs=NT) as m1_pool,
            tc.tile_pool(name="pp", bufs=4) as p_pool,
            tc.tile_pool(name="comb", bufs=4) as comb_pool,
            tc.tile_pool(name="outs", bufs=NT) as out_pool,
            tc.tile_pool(name="gsbp", bufs=NT) as gsb_pool,
            tc.tile_pool(name="dram", bufs=1, space="DRAM") as dram_pool,
        ):
            # ---- constant / weight loads ----
            xsT_sb = singles.tile([128, 2, N], f32)
            nc.sync.dma_start(xsT_sb, xsT.rearrange("(ko p) n -> p ko n", p=128))
            xtT_sb = singles.tile([128, 2, N], f32)
            nc.sync.dma_start(xtT_sb, xtT.rearrange("(ko p) n -> p ko n", p=128))
            wes_sb = singles.tile([128, 2, 260], f32)
            nc.sync.dma_start(wes_sb, wes.rearrange("(ko p) n -> p ko n", p=128))
            wbt_sb = singles.tile([128, 2, 4], f32)
            nc.sync.dma_start(wbt_sb, wbt.rearrange("(ko p) n -> p ko n", p=128))
            maskp_sb = singles.tile([128, NT], f32)
            nc.sync.dma_start(maskp_sb, maskp)
            bias_bc = singles.tile([128, 256], f32)
            nc.gpsimd.dma_start(bias_bc, biasrow.broadcast_to([128, 256]))

            em_dram = dram_pool.tile([N, N], f16)

            # ---- phase A: h_src matmul + R build (source side) ----
            r_tiles = []
            a_tiles = []
            for st in range(NT):
                ps = psum_pool.tile([128, 512], f32, tag="ps")
                for ko in range(2):
                    nc.tensor.matmul(
                        ps[:, :260],
                        lhsT=xsT_sb[:, ko, ts(st, 128)],
                        rhs=wes_sb[:, ko, :],
                        start=(ko == 0),
                        stop=(ko == 1),
                    )
                a_sb = a_pool.tile([128, 4], f32)
                nc.scalar.activation(a_sb, ps[:, 256:260], Act.Identity)
                F = f_pool.tile([128, 2, 4], f32)
                nc.scalar.activation(F[:, 0, :], ps[:, 256:260], Act.Exp)
                nc.scalar.activation(F[:, 1, :], ps[:, 256:260], Act.Exp, scale=0.2)
                # fold source-side mask into the F scales (masks both num & den)
                nc.vector.tensor_scalar(
                    F[:, :, :], F[:, :, :], maskp_sb[:, st : st + 1], None, Alu.mult
                )
                R = r_pool.tile([128, 4, 130], f16)
                for h in range(4):
                    nc.scalar.activation(
                        R[:, h, 0:64], ps[:, h * 64 : (h + 1) * 64],
                        Act.Identity, scale=F[:, 0, h : h + 1],
                    )
                    nc.scalar.activation(
                        R[:, h, 65:129], ps[:, h * 64 : (h + 1) * 64],
                        Act.Identity, scale=F[:, 1, h : h + 1],
                    )
                nc.vector.tensor_copy(out=R[:, :, 64], in_=F[:, 0, :])
                nc.vector.tensor_copy(out=R[:, :, 129], in_=F[:, 1, :])
                r_tiles.append(R)
                a_tiles.append(a_sb)

            # ---- phase A2: target side (r scales + u vectors) ----
            r_sb_tiles = []
            for tt in range(NT):
                ps = psum_pool.tile([128, 512], f32, tag="ps")
                for ko in range(2):
                    nc.tensor.matmul(
                        ps[:, 0:4],
                        lhsT=xtT_sb[:, ko, ts(tt, 128)],
                        rhs=wbt_sb[:, ko, :],
                        start=(ko == 0),
                        stop=(ko == 1),
                    )
                r_sb = a_pool.tile([128, 4], f32, tag="rsb")
                nc.scalar.activation(r_sb, ps[:, 0:4], Act.Exp, scale=0.8)
                r_sb_tiles.append(r_sb)

            u_sb = singles.tile([4, N], f16)
            for half in range(2):
                ps = psum_pool.tile([128, 512], f32, tag="ps")
                for ko in range(2):
                    nc.tensor.matmul(
                        ps[0:4, 0:512],
                        lhsT=wbt_sb[:, ko, :],
                        rhs=xtT_sb[:, ko, ds(half * 512, 512)],
                        start=(ko == 0),
                        stop=(ko == 1),
                    )
                nc.scalar.activation(
                    u_sb[:, half * 512 : (half + 1) * 512], ps[0:4, 0:512], Act.Identity
                )
            u_dram = dram_pool.tile([4, N], f16)
            nc.sync.dma_start(u_dram, u_sb)
            u_pair = []
            for pp in range(2):
                up = singles.tile([128, 2, N], f16, tag=f"upair{pp}")
                for i in range(2):
                    h = 2 * pp + i
                    nc.gpsimd.dma_start(
                        out=up[:, i, :], in_=u_dram[h : h + 1, :].broadcast_to([128, N])
                    )
                u_pair.append(up)

            # ---- phase B: edge mask em = (adj != 0), transposed via DRAM ----
            for tt in range(NT):
                adj_t = adj_pool.tile([128, N], f32)
                nc.sync.dma_start(adj_t, adj[ts(tt, 128), :])
                em16 = em16_pool.tile([128, N], f16)
                nc.vector.tensor_scalar(em16, adj_t, 0.0, None, Alu.not_equal)
                nc.sync.dma_start(em_dram[ts(tt, 128), :], em16)
            emT_tiles = []
            for st in range(NT):
                emT = emT_pool.tile([128, N], f16)
                nc.sync.dma_start_transpose(emT, em_dram[:, ts(st, 128)])
                emT_tiles.append(emT)

            # ---- phase C: two head-pass pipelines ----
            g_sb_tiles = [None] * NT
            out_tiles = [
                out_pool.tile([128, 256], f32, name=f"outt{t}", tag="outt") for t in range(NT)
            ]

            for p in range(2):
                heads = (2 * p, 2 * p + 1)
                # masks M1 for this head pair, all 8 s-tiles (resident)
                m1_tiles = []
                for st in range(NT):
                    m1 = m1_pool.tile([128, 2, N], f16)
                    for i, h in enumerate(heads):
                        pt = p_pool.tile([128, N], f16)
                        nc.vector.tensor_scalar(
                            pt, u_pair[p][:, i, :],
                            a_tiles[st][:, h : h + 1], 0.0, Alu.add, Alu.is_ge,
                        )
                        nc.vector.tensor_tensor(m1[:, i, :], pt, emT_tiles[st], Alu.mult)
                    m1_tiles.append(m1)

                tgroups = [[0, 1, 2, 3], [4, 5, 6, 7]] if p == 0 else [list(range(NT))]
                for g in tgroups:
                    psm = {}
                    psg = {}
                    for t in g:
                        psm[t] = psum_pool.tile([128, 512], f32, name=f"psm{p}_{t}", tag="ps")
                        if p == 0:
                            psg[t] = psum_pool.tile([128, 512], f32, name=f"psg{t}", tag="ps")
                    for i, h in enumerate(heads):
                        for st in range(NT):
                            for t in g:
                                nc.tensor.matmul(
                                    psm[t][:, i * 130 : (i + 1) * 130],
                                    lhsT=m1_tiles[st][:, i, ts(t, 128)],
                                    rhs=r_tiles[st][:, h, :],
                                    start=(st == 0),
                                    stop=(st == NT - 1),
                                )
                    if p == 0:
                        for st in range(NT):
                            for t in g:
                                nc.tensor.matmul(
                                    psg[t][:, 0:260],
                                    lhsT=emT_tiles[st][:, ts(t, 128)],
                                    rhs=r_tiles[st][:, :, 65:130],
                                    start=(st == 0),
                                    stop=(st == NT - 1),
                                )
                    # combine per t-tile
                    for t in g:
                        if p == 0:
                            g_sb = gsb_pool.tile([128, 4, 65], f32, tag="gsb")
                            nc.scalar.activation(
                                g_sb.rearrange("p a b -> p (a b)"),
                                psg[t][:, 0:260], Act.Identity,
                            )
                            g_sb_tiles[t] = g_sb
                        V = comb_pool.tile([128, 2, 65], f32, tag="vt")
                        for i, h in enumerate(heads):
                            nc.scalar.activation(
                                V[:, i, :], psm[t][:, i * 130 : i * 130 + 65],
                                Act.Identity, scale=r_sb_tiles[t][:, h : h + 1],
                            )
                        A2 = comb_pool.tile([128, 2, 65], f32, tag="a2")
                        psm_r = psm[t][:, 0:260].rearrange("p (i c) -> p i c", i=2)
                        nc.scalar.activation(A2, psm_r[:, :, 65:130], Act.Identity)
                        W = comb_pool.tile([128, 2, 65], f32, tag="wt")
                        nc.vector.tensor_tensor(
                            W, V, g_sb_tiles[t][:, 2 * p : 2 * p + 2, :], Alu.add
                        )
                        nc.vector.tensor_tensor(W, W, A2, Alu.subtract)
                        dent = comb_pool.tile([128, 2], f32, tag="dent")
                        nc.vector.tensor_scalar(dent, W[:, :, 64], EPS, None, Alu.add)
                        nc.vector.reciprocal(dent, dent)
                        for i, h in enumerate(heads):
                            nc.vector.tensor_scalar(
                                out_tiles[t][:, h * 64 : (h + 1) * 64],
                                W[:, i, 0:64], dent[:, i : i + 1], None, Alu.mult,
                            )
                        if p == 1:
                            nc.vector.tensor_tensor(
                                out_tiles[t], out_tiles[t], bias_bc, Alu.add
                            )
                            nc.vector.tensor_scalar(
                                out_tiles[t], out_tiles[t],
                                maskp_sb[:, t : t + 1], None, Alu.mult,
                            )
                            nc.sync.dma_start(out[ts(t, 128), :], out_tiles[t])

    nc.compile()
    return nc


def host_prep(x_source, x_target, adj, mask, W_src, W_tgt, att_src, att_tgt, bias):
    """Per-core input maps (layout prep only: transposes / weight folding)."""
    x_source = np.asarray(x_source, dtype=np.float32)
    x_target = np.asarray(x_target, dtype=np.float32)
    adj = np.ascontiguousarray(np.asarray(adj, dtype=np.float32))
    mask = np.asarray(mask)
    W_src = np.asarray(W_src, dtype=np.float32)
    W_tgt = np.asarray(W_tgt, dtype=np.float32)
    att_src = np.asarray(att_src, dtype=np.float32)
    att_tgt = np.asarray(att_tgt, dtype=np.float32)
    bias = np.asarray(bias, dtype=np.float32)

    w_a = np.einsum(
        "hdc,hd->ch", W_src.astype(np.float64).reshape(H, D, C), att_src.astype(np.float64)
    ).astype(np.float32)
    w_b = np.einsum(
        "hdc,hd->ch", W_tgt.astype(np.float64).reshape(H, D, C), att_tgt.astype(np.float64)
    ).astype(np.float32)
    wes = np.ascontiguousarray(np.concatenate([W_src.T, w_a], axis=1))  # (256, 260)
    wbt = np.ascontiguousarray(w_b)  # (256, 4)
    biasrow = np.ascontiguousarray(bias.reshape(1, 256))

    in_maps = []
    for b in range(B):
        maskp = (
            mask[b].astype(np.float32).reshape(NT, 128).T.copy()
        )  # (128, NT), p-inner
        in_maps.append(
            {
                "xsT": np.ascontiguousarray(x_source[b].T),
                "xtT": np.ascontiguousarray(x_target[b].T),
                "adj": adj[b],
                "maskp": maskp,
                "wes": wes,
                "wbt": wbt,
                "biasrow": biasrow,
            }
        )
    return in_maps


def get_nc():
    if "nc" not in _CACHED:
        _install_neff_cache()
        _CACHED["nc"] = build_nc()
    return _CACHED["nc"]


def kernel(**inputs) -> np.ndarray:
    from concourse.bass_utils import run_bass_kernel_spmd

    nc = get_nc()
    in_maps = host_prep(**inputs)
    res = run_bass_kernel_spmd(nc, in_maps, core_ids=list(range(B)))
    return np.stack([r["out"] for r in res.results]).astype(np.float32)



# revision 2
# speedup vs baseline: 9.8345x; 9.8345x over previous
# Trainium2 Bass kernel for DenseBipartiteGAT (B=8, N=1024, C=256, H=4, D=64).
#
# Math: scores[t,s,h] = lrelu(a_tgt[t,h] + a_src[s,h], 0.2), masked softmax over s,
#       out[t] = sum_s attn * h_src.
# Factorization: exp(lrelu(u+v)) = e^u e^v if u+v>=0 else e^.2u e^.2v. Dividing
# num/den cancels e^{.2u}, leaving r=e^{.8u}:
#   out = (r*A1 + (G - A2)) / (r*A1d + (G - A2)d + eps)
# with A1 = sum_s m1*F1*hsrc, A2 = sum_s m1*F2*hsrc, G = sum_s em*F2*hsrc,
# m1 = em*[u+v>=0], F1 = e^v*mask_s, F2 = e^{.2v}*mask_s.
#
# Rewrite with the complement mask mx = em*[u+v<0] (m1 = em - mx):
#   A1     = em@R1 - mx@R1
#   G - A2 = mx@R2
# Using signed encodings ptxn = -[u<-a], mxn = ptxn*em (0/-1), em (0/1) and
# signed R-regions R1p = F1*hsrc, R2n = -F2*hsrc (den cols F1 / -F2), a single
# PSUM tile [j,h,65] accumulates  j0: em@R1p + mxn@R1p = A1(+den1),
#                                 j1: mxn@R2n = (G-A2)(+den).
# Combine: W = r*psm[j0] + psm[j1]; out = W[:, :64] * mask/(W[:,64]+eps) + bias.
#
# Per-core O(N^2) vector work: em (1 op/st), ptxn (1 op/(st,h)), mxn (1 op/(st,h)).
# Everything else is PE matmuls (f32r for the fp32 input projections).
#
# Sharding: data-parallel over batch B across the 8 cores (1 batch element each).

import hashlib
import os
import shutil

import numpy as np

B, N, C, H, D = 8, 1024, 256, 4, 64
NT = N // 128  # 8 tiles of 128 along s or t
EPS = 1e-12

_CACHED = {}


def _install_neff_cache():
    """Content-addressed NEFF cache: walrus compile is ~8min, cache by BIR hash."""
    import concourse.bass2jax as b2j
    import concourse.bass_utils as bu

    if getattr(b2j, "_neff_cache_installed", False):
        return
    cache_dir = os.environ.get("NEFF_CACHE_DIR", "/tmp/neff_cache")
    os.makedirs(cache_dir, exist_ok=True)
    orig = bu.compile_bir_kernel

    def cached_compile(bir_json: bytes, tmpdir: str, neff_name="file.neff") -> str:
        key = hashlib.sha256(bir_json).hexdigest()
        cpath = os.path.join(cache_dir, f"{key}.neff")
        opath = os.path.join(tmpdir, neff_name)
        if os.path.exists(cpath):
            shutil.copy(cpath, opath)
            return opath
        neff = orig(bir_json, tmpdir, neff_name)
        try:
            shutil.copy(neff, cpath)
        except OSError:
            pass
        return neff

    bu.compile_bir_kernel = cached_compile
    b2j.compile_bir_kernel = cached_compile
    b2j._neff_cache_installed = True


def build_nc():
    """Build the Bass program (one core's work; SPMD across 8 cores)."""
    import concourse.tile as tile
    import concourse.mybir as mybir
    from concourse import bacc
    from concourse.bass import ts, ds

    f32 = mybir.dt.float32
    f32r = mybir.dt.float32r
    f16 = mybir.dt.float16
    Alu = mybir.AluOpType
    Act = mybir.ActivationFunctionType

    nc = bacc.Bacc("TRN2", target_bir_lowering=False, debug=False, num_devices=B)

    xsT = nc.dram_tensor("xsT", (C, N), f16, kind="ExternalInput").ap()
    xtT = nc.dram_tensor("xtT", (C, N), f16, kind="ExternalInput").ap()
    adjTh = nc.dram_tensor("adjTh", (N, N), f16, kind="ExternalInput").ap()
    maskp = nc.dram_tensor("maskp", (128, NT), f32, kind="ExternalInput").ap()
    maskln = nc.dram_tensor("maskln", (128, NT), f32, kind="ExternalInput").ap()
    wes = nc.dram_tensor("wes", (C, 260), f16, kind="ExternalInput").ap()
    wbt = nc.dram_tensor("wbt", (C, 4), f16, kind="ExternalInput").ap()
    biasrow = nc.dram_tensor("biasrow", (1, 256), f32, kind="ExternalInput").ap()
    out = nc.dram_tensor("out", (N, 256), f32, kind="ExternalOutput").ap()

    with tile.TileContext(nc) as tc:
        with (
            tc.tile_pool(name="singles", bufs=1) as singles,
            tc.tile_pool(name="pch", bufs=8, space="PSUM") as pch,
            tc.tile_pool(name="adjs", bufs=3) as adj_pool,
            tc.tile_pool(name="emp", bufs=NT) as em_pool,
            tc.tile_pool(name="rp", bufs=NT) as r_pool,
            tc.tile_pool(name="fx", bufs=2) as f_pool,
            tc.tile_pool(name="ab", bufs=NT) as a_pool,
            tc.tile_pool(name="mx", bufs=4 * NT) as mx_pool,
            tc.tile_pool(name="ptp", bufs=3) as pt_pool,
            tc.tile_pool(name="wt", bufs=4) as w_pool,
            tc.tile_pool(name="outs", bufs=2) as out_pool,
            tc.tile_pool(name="dram", bufs=1, space="DRAM") as dram_pool,
        ):
            # ---- loads. Act queue: xtT only (u critical path); SP: the rest+adjT;
            # gpsimd: u roundtrip + bias.
            xtT_sb = singles.tile([128, 2, N], f16)
            nc.scalar.dma_start(xtT_sb, xtT.rearrange("(ko p) n -> p ko n", p=128))
            xsT_sb = singles.tile([128, 2, N], f16)
            nc.sync.dma_start(xsT_sb, xsT.rearrange("(ko p) n -> p ko n", p=128))
            wes_sb = singles.tile([128, 2, 260], f16)
            nc.sync.dma_start(wes_sb, wes.rearrange("(ko p) n -> p ko n", p=128))
            wbt_sb = singles.tile([128, 2, 4], f16)
            nc.scalar.dma_start(wbt_sb, wbt.rearrange("(ko p) n -> p ko n", p=128))
            maskp_sb = singles.tile([128, NT], f32)
            nc.sync.dma_start(maskp_sb, maskp)
            maskln_sb = singles.tile([128, NT], f32)
            nc.sync.dma_start(maskln_sb, maskln)
            bias_bc = singles.tile([128, 256], f32)
            nc.gpsimd.dma_start(bias_bc, biasrow.broadcast_to([128, 256]))

            # ---- u^T = wbt^T @ xtT in 256-col chunks -> DRAM -> broadcast ----
            u_sb = singles.tile([4, N], f16)
            for q in range(4):
                psu = pch.tile([128, 6, 65], f32, tag="px", bufs=8, name="psu")
                psuf = psu.rearrange("p a b -> p (a b)")
                for ko in range(2):
                    nc.tensor.matmul(
                        psuf[0:4, 0:256],
                        lhsT=wbt_sb[:, ko, :],
                        rhs=xtT_sb[:, ko, ds(q * 256, 256)],
                        start=(ko == 0),
                        stop=(ko == 1),
                    )
                nc.scalar.activation(
                    u_sb[:, q * 256 : (q + 1) * 256], psuf[0:4, 0:256], Act.Identity
                )
            u_dram = dram_pool.tile([4, N], f16)
            nc.gpsimd.dma_start(u_dram, u_sb)
            u_bc = singles.tile([128, 4, N], f16)
            ones_sb = singles.tile([1, 128], f16)
            nc.vector.memset(ones_sb, 1.0)
            for q in range(4):
                psb = pch.tile([128, 6, 65], f32, tag="px", bufs=8, name="psb")
                psbf = psb.rearrange("p a b -> p (a b)")
                nc.tensor.matmul(
                    psbf[:, 0:256],
                    lhsT=ones_sb,
                    rhs=u_sb[0:1, ds(q * 256, 256)],
                    start=True,
                    stop=True,
                )
                nc.scalar.activation(
                    u_bc[:, 0, q * 256 : (q + 1) * 256], psbf[:, 0:256], Act.Identity
                )

            # ---- phase A per s-tile: hsrc matmul -> F exps -> R build -> a_neg ----
            R_tiles = []
            an_tiles = []
            for st in range(NT):
                psx = pch.tile([128, 6, 65], f32, tag="px", bufs=8, name="psx")
                ps = psx.rearrange("p a b -> p (a b)")[:, 0:260]
                for ko in range(2):
                    nc.tensor.matmul(
                        ps,
                        lhsT=xsT_sb[:, ko, ts(st, 128)],
                        rhs=wes_sb[:, ko, :],
                        start=(ko == 0),
                        stop=(ko == 1),
                    )
                lnm = maskln_sb[:, st : st + 1]
                # a_neg = -a_src (scalar operand of the ptxn compare)
                an = a_pool.tile([128, 4], f32, name=f"an{st}", tag="an")
                nc.scalar.activation(an, ps[:, 256:260], Act.Identity, scale=-1.0)
                an_tiles.append(an)
                Fx = f_pool.tile([128, 2, 4], f32, tag="fx")
                nc.scalar.activation(Fx[:, 0, :], ps[:, 256:260], Act.Exp, bias=lnm)
                nc.scalar.activation(
                    Fx[:, 1, :], ps[:, 256:260], Act.Exp, bias=lnm, scale=0.2
                )
                # negate F2 in place (R2 region is stored negated)
                nc.gpsimd.tensor_scalar(Fx[:, 1, :], Fx[:, 1, :], -1.0, None, Alu.mult)
                R = r_pool.tile([128, 2, 4, 65], f16, name=f"R{st}", tag="R")
                ps4 = ps[:, 0:256].rearrange("p (h d) -> p h d", h=4)
                for j in range(2):
                    nc.gpsimd.tensor_tensor(
                        R[:, j, :, 0:64],
                        ps4,
                        Fx[:, j, :].unsqueeze(2).broadcast_to([128, 4, 64]),
                        Alu.mult,
                    )
                nc.gpsimd.tensor_copy(out=R[:, :, :, 64], in_=Fx)
                R_tiles.append(R)

            # ---- r_sb[t_part, h] = exp(0.8 * a_tgt) per t-tile ----
            r_sb_tiles = []
            for tt in range(NT):
                psr = pch.tile([128, 6, 65], f32, tag="px", bufs=8, name="psr")
                psrf = psr.rearrange("p a b -> p (a b)")
                for ko in range(2):
                    nc.tensor.matmul(
                        psrf[:, 0:4],
                        lhsT=xtT_sb[:, ko, ts(tt, 128)],
                        rhs=wbt_sb[:, ko, :],
                        start=(ko == 0),
                        stop=(ko == 1),
                    )
                r_sb = a_pool.tile([128, 4], f32, name=f"rsb{tt}", tag="rsb")
                nc.scalar.activation(r_sb, psrf[:, 0:4], Act.Exp, scale=0.8)
                r_sb_tiles.append(r_sb)

            # ---- adjT loads (SP queue; u_bc h1-3 DMA slotted after #2) ----
            adjT_tiles = []
            for st in range(NT):
                adjT = adj_pool.tile([128, N], f16, tag="adjT", bufs=NT)
                nc.sync.dma_start(adjT, adjTh[ts(st, 128), :])
                adjT_tiles.append(adjT)
                if st == 2:
                    # heads 1-3 of u broadcast in one DMA (h0 via PE above)
                    nc.sync.dma_start(
                        u_bc[:, 1:4, :],
                        u_dram[1:4, :].unsqueeze(0).broadcast_to([128, 3, N]),
                    )

            # ---- em (0/1) + masks mxn = -[u < -a] * em  (0/-1) ----
            # DVE order: em 0-2, ptxn h0, em 3-7, then per-h mask stream.
            em_tiles = [None] * NT

            def emit_em(st):
                em = em_pool.tile([128, N], f16, name=f"em{st}", tag="em")
                nc.vector.tensor_scalar(em, adjT_tiles[st], 0.0, None, Alu.not_equal)
                em_tiles[st] = em

            mx_tiles = [[None] * 4 for _ in range(NT)]

            def emit_ptx(st, h):
                ptx = pt_pool.tile([128, N], f16, tag="ptx", bufs=12)
                nc.vector.tensor_scalar(
                    ptx, u_bc[:, h, :], an_tiles[st][:, h : h + 1], -1.0,
                    Alu.is_lt, Alu.mult,
                )
                return ptx

            def emit_mx(st, h, ptx):
                mx = mx_pool.tile([128, N], f16, name=f"mx{st}_{h}", tag="mx")
                if (st * 4 + h) % 16 < 9:
                    nc.gpsimd.tensor_tensor(mx, ptx, em_tiles[st], Alu.mult)
                else:
                    nc.vector.tensor_tensor(mx, ptx, em_tiles[st], Alu.mult)
                mx_tiles[st][h] = mx

            for st in range(3):
                emit_em(st)
            ptx_h0 = [emit_ptx(st, 0) for st in range(NT)]
            for st in range(3, NT):
                emit_em(st)
            for st in range(NT):
                emit_mx(st, 0, ptx_h0[st])

            # ---- chains: head-pair two-phase over all 8 t-tiles ----
            # px tile per (t, head-pair): [ha:j0,j1 | hb:j0,j1 | emE ha,hb]
            # j0 -= mx@R1p, j1 = mx@R2, emE = em@R1p
            pX_tiles = [None] * NT
            W_tiles = [None] * NT

            def emit_em_chain(t, hp):
                pX = pch.tile([128, 6, 65], f32, name=f"psm{hp}_{t}", tag="px", bufs=8)
                pX_tiles[t] = pX
                for st in range(NT):
                    nc.tensor.matmul(
                        pX[:, 4:6, :].rearrange("p a b -> p (a b)"),
                        lhsT=em_tiles[st][:, ts(t, 128)],
                        rhs=R_tiles[st][:, 0, 2 * hp : 2 * hp + 2, :],
                        start=(st == 0),
                        stop=(st == NT - 1),
                    )

            def emit_mx_chain(t, h):
                pX = pX_tiles[t]
                for st in range(NT):
                    nc.tensor.matmul(
                        pX[:, 2 * (h % 2) : 2 * (h % 2) + 2, :].rearrange(
                            "p a b -> p (a b)"
                        ),
                        lhsT=mx_tiles[st][h][:, ts(t, 128)],
                        rhs=R_tiles[st][:, :, h, :],
                        start=False,
                        stop=False,
                        skip_group_check=True,
                    )

            def emit_W(t, hp):
                # W[ha,hb] = r*(emE + mx_j0) + mx_j1 ; frees the px tile
                if hp == 0:
                    W_tiles[t] = w_pool.tile(
                        [128, 4, 65], f32, name=f"W{t}", tag="W", bufs=NT
                    )
                W = W_tiles[t]
                pX = pX_tiles[t]
                for i in range(2):
                    h = 2 * hp + i
                    nc.gpsimd.tensor_tensor(
                        W[:, h, :], pX[:, 4 + i, :], pX[:, 2 * i, :], Alu.add
                    )
                    nc.gpsimd.scalar_tensor_tensor(
                        W[:, h, :], W[:, h, :], r_sb_tiles[t][:, h : h + 1],
                        pX[:, 2 * i + 1, :], Alu.mult, Alu.add,
                    )

            def emit_final(t):
                W = W_tiles[t]
                dent = w_pool.tile([128, 4], f32, tag="dent")
                nc.vector.tensor_scalar(dent, W[:, :, 64], EPS, None, Alu.add)
                nc.vector.reciprocal(dent, dent)
                nc.gpsimd.tensor_scalar(
                    dent, dent, maskp_sb[:, t : t + 1], None, Alu.mult
                )
                ot = out_pool.tile([128, 256], f32, tag="ot")
                ov = ot.rearrange("p (h d) -> p h d", h=4)
                eng = nc.vector if t % 2 == 0 else nc.gpsimd
                eng.tensor_tensor(
                    ov, W[:, :, 0:64],
                    dent.unsqueeze(2).broadcast_to([128, 4, 64]), Alu.mult,
                )
                eng.scalar_tensor_tensor(
                    ot, bias_bc, maskp_sb[:, t : t + 1], ot, Alu.mult, Alu.add
                )
                nc.sync.dma_start(out[ts(t, 128), :], ot)

            for t in range(NT):
                emit_em_chain(t, 0)
            for st in range(NT):
                emit_mx(st, 1, emit_ptx(st, 1))
            for t in range(NT):
                emit_mx_chain(t, 0)
            for t in range(NT):
                emit_mx_chain(t, 1)
            for t in range(NT):
                emit_W(t, 0)
            for t in range(NT):
                emit_em_chain(t, 1)
            for st in range(NT):
                emit_mx(st, 2, emit_ptx(st, 2))
            for t in range(NT):
                emit_mx_chain(t, 2)
            for st in range(NT):
                emit_mx(st, 3, emit_ptx(st, 3))
            for t in range(NT):
                emit_mx_chain(t, 3)
            for t in range(NT):
                emit_W(t, 1)
                emit_final(t)

    nc.compile()
    return nc


def host_prep(x_source, x_target, adj, mask, W_src, W_tgt, att_src, att_tgt, bias):
    """Per-core input maps (layout prep only: transposes / views / weight folding)."""
    x_source = np.asarray(x_source, dtype=np.float32)
    x_target = np.asarray(x_target, dtype=np.float32)
    adj = np.ascontiguousarray(np.asarray(adj, dtype=np.float32))
    mask = np.asarray(mask)
    W_src = np.asarray(W_src, dtype=np.float32)
    W_tgt = np.asarray(W_tgt, dtype=np.float32)
    att_src = np.asarray(att_src, dtype=np.float32)
    att_tgt = np.asarray(att_tgt, dtype=np.float32)
    bias = np.asarray(bias, dtype=np.float32)

    w_a = np.einsum(
        "hdc,hd->ch", W_src.astype(np.float64).reshape(H, D, C), att_src.astype(np.float64)
    ).astype(np.float32)
    w_b = np.einsum(
        "hdc,hd->ch", W_tgt.astype(np.float64).reshape(H, D, C), att_tgt.astype(np.float64)
    ).astype(np.float32)
    wes = np.ascontiguousarray(np.concatenate([W_src.T, w_a], axis=1).astype(np.float16))  # (256, 260)
    wbt = np.ascontiguousarray(w_b.astype(np.float16))  # (256, 4)
    biasrow = np.ascontiguousarray(bias.reshape(1, 256))

    in_maps = []
    for b in range(B):
        mb = mask[b].astype(np.float32)
        maskp = mb.reshape(NT, 128).T.copy()  # (128, NT), p-inner
        maskln = np.where(mb > 0, 0.0, -60.0).astype(np.float32).reshape(NT, 128).T.copy()
        # hi 2 bytes of each f32: zero iff (virtually certainly) adj == 0.
        adjTh = np.ascontiguousarray(adj[b].view(np.float16)[:, 1::2].T)  # (Ns, Nt)
        in_maps.append(
            {
                "xsT": np.ascontiguousarray(x_source[b].T.astype(np.float16)),
                "xtT": np.ascontiguousarray(x_target[b].T.astype(np.float16)),
                "adjTh": adjTh,
                "maskp": maskp,
                "maskln": maskln,
                "wes": wes,
                "wbt": wbt,
                "biasrow": biasrow,
            }
        )
    return in_maps


def get_nc():
    if "nc" not in _CACHED:
        _install_neff_cache()
        _CACHED["nc"] = build_nc()
    return _CACHED["nc"]


def kernel(**inputs) -> np.ndarray:
    from concourse.bass_utils import run_bass_kernel_spmd

    nc = get_nc()
    in_maps = host_prep(**inputs)
    res = run_bass_kernel_spmd(nc, in_maps, core_ids=list(range(B)))
    return np.stack([r["out"] for r in res.results]).astype(np.float32)
